# revision 12
# baseline (speedup 1.0000x reference)
"""GCN layer kernel for trn2, 8-core SPMD — v5.1.

out = relu(D^-1/2 (A+I) D^-1/2 X W + b) = relu((A_hat X) W + b)

Architecture (per core, dst-sharded; GROUP=7 dst tiles per gather group, src
space in 4 chunks of 25088 rows for int16 gather indices):
- Self-loops are excluded from the edge list. Their contribution
  (dis_d^2 * x_d) enters per tile as matmul(lhsT=cast(x_tile)*dis^2,
  rhs=identity): the ACT cast's per-partition `scale` applies dis^2 free,
  and the x tile is a cheap sequential load. This avoids the self-loop
  block inflating every (tile,chunk) segment's max-over-cores count.
- Edge segments per (tile,chunk) are padded to the max count over the 8
  cores (core-uniform schedule), not to x128; blocks may cross tile
  boundaries. A block containing a transition carries rank-1 edges
  (enc = dstloc + 128); those blocks are permuted to the span end so the
  rank-1 one-hot builds are small contiguous flip-layout DVE passes.
- One dma_gather per (group, chunk) fetches raw fp32 x rows (512B each).
- ACT casts gathers fp32->fp16; DVE builds M_norm = is_equal * norm in the
  flip layout m[p, d*SMAX+j] (2x mode); PE accumulates
  accT[f,d] += sum_e norm_e x[src_e]^T via matmul(lhsT=G_j, rhs=M[:,:,j]).
- Per tile: ACT copies accT to fp16; PE applies W + rank-1 bias; ACT relu;
  one batched store per group.
"""
import numpy as np

import concourse.bass as bass
import concourse.mybir as mybir
import concourse.tile as tile
from concourse import library_config
from concourse.bass_utils import run_bass_kernel_spmd
from concourse.library_overlay import lower_extended_insts
from concourse.masks import make_identity

# ---- walrus workaround: <=1 sync wait per instruction ----
def _install_tile_patches():
    K = 1

    def _split_waits(tc, ordered):
        nc = tc.nc
        for insts in ordered.values():
            out = []
            for inst in insts:
                si = inst.sync_info
                waits = list(si.on_wait) if si is not None else []
                if len(waits) > K:
                    for i in range(0, len(waits) - K, K):
                        out.append(mybir.InstNoOp(
                            name=nc.get_next_instruction_name(),
                            engine=inst.engine, ins=[], outs=[],
                            sync_info=mybir.SyncInfo(on_wait=waits[i:i+K], on_update=[]),
                        ))
                    inst.sync_info = mybir.SyncInfo(
                        on_wait=waits[len(waits) - K:], on_update=list(si.on_update))
                out.append(inst)
            insts[:] = out

    if getattr(tile.TileContext, "_ant_waitsplit", False):
        return
    orig_lower = tile.TileContext._lower_ordered_insts

    def lower(self, ordered):
        _split_waits(self, ordered)
        return orig_lower(self, ordered)

    def drain(self, tick_clock, wait_clock):
        probe = self.nc.sync.nop(nofuse=True, hint="pre_drain_waits")
        wait_clock.add_sem_waits(probe.ins, tile.ScopedClock({None: tick_clock.global_clock}))
        waits = list(probe.ins.sync_info.on_wait)
        probe.ins.sync_info = mybir.SyncInfo(on_wait=waits[:K], on_update=[])
        for i in range(K, len(waits), K):
            n = self.nc.sync.nop(nofuse=True, hint="pre_drain_waits")
            n.ins.sync_info = mybir.SyncInfo(on_wait=waits[i:i+K], on_update=[])
        self.nc.sync.drain()
        self.nc.all_engine_barrier()
        popped = self.nc._tile_sem_poison_stack.pop()
        assert popped is self._sem_poison
        self.nc.clear_and_free_semaphores(list(self.sems.allocated().values()))
        self.nc.all_engine_barrier()

    tile.TileContext._lower_ordered_insts = lower
    tile.TileContext._drain_and_barrier = drain
    tile.TileContext._ant_waitsplit = True



N = 100000
D = 128
NCORES = 8
P = 128
VPAD = 100352
SH = VPAD // NCORES      # 12544
NT_SH = SH // P          # 98
NCHUNK = 4
CHUNK = VPAD // NCHUNK   # 25088
GROUP = 7
NG = NT_SH // GROUP      # 14

F16 = mybir.dt.float16
F32 = mybir.dt.float32
I16 = mybir.dt.int16


def _host_prep(edge_index):
    """Returns (idx_tiles, enc_tiles, norm_tiles, dis2_tiles, sched)."""
    src = edge_index[0].astype(np.int64)
    dst = edge_index[1].astype(np.int64)

    deg = np.bincount(dst, minlength=VPAD).astype(np.float64) + 1.0
    dis = 1.0 / np.sqrt(deg)
    norm = (dis[src] * dis[dst]).astype(np.float32)
    dis2 = (1.0 / deg).astype(np.float32)
    dis2_tiles = [
        np.ascontiguousarray(
            dis2[c * SH:(c + 1) * SH].reshape(NT_SH, P).T.astype(np.float32))
        for c in range(NCORES)
    ]

    c_of = dst // SH
    t_loc = (dst % SH) // P
    g_of = t_loc // GROUP
    tg_of = t_loc % GROUP
    q_of = src // CHUNK
    key = (((c_of * NG + g_of) * NCHUNK) + q_of) * GROUP + tg_of
    NKEYS = NCORES * NG * NCHUNK * GROUP

    order = np.lexsort((src, key))
    src_s = src[order]
    dst_s = dst[order]
    nrm_s = norm[order]
    key_s = key[order]
    q_s = q_of[order]

    cnt = np.bincount(key_s, minlength=NKEYS).reshape(NCORES, NG, NCHUNK, GROUP)
    L = np.maximum(cnt.max(axis=0), P)            # [NG, NCHUNK, GROUP]
    seg_off = np.zeros((NG, NCHUNK, GROUP), dtype=np.int64)
    seg_off[:, :, 1:] = np.cumsum(L, axis=2)[:, :, :-1]
    span_raw = L.sum(axis=2)
    SB = (span_raw + P - 1) // P

    nb1 = np.zeros((NG, NCHUNK), dtype=np.int64)
    t0_lists = [[None] * NCHUNK for _ in range(NG)]
    perm_maps = [[None] * NCHUNK for _ in range(NG)]
    for g in range(NG):
        for q in range(NCHUNK):
            sb = int(SB[g, q])
            offs = seg_off[g, q]
            raw_t0 = np.searchsorted(offs, np.arange(sb) * P, side="right") - 1
            bset = sorted({int(offs[t]) // P for t in range(1, GROUP)
                           if offs[t] % P != 0})
            nb = len(bset)
            nonb = [bk for bk in range(sb) if bk not in bset]
            old_order = nonb + bset
            perm = np.empty(sb, dtype=np.int64)
            for newj, oldb in enumerate(old_order):
                perm[oldb] = newj
            nb1[g, q] = nb
            t0_lists[g][q] = [int(raw_t0[oldb]) for oldb in old_order]
            perm_maps[g][q] = perm

    span_blk_base = np.zeros((NG, NCHUNK), dtype=np.int64)
    span_blk_base.reshape(-1)[1:] = np.cumsum(SB.reshape(-1))[:-1]
    NBLK = int(SB.sum())
    NIDX = NBLK * P

    seg_start = np.zeros(NKEYS + 1, dtype=np.int64)
    np.cumsum(np.bincount(key_s, minlength=NKEYS), out=seg_start[1:])
    rank_in_seg = np.arange(len(key_s), dtype=np.int64) - seg_start[key_s]
    gqt = key_s % (NG * NCHUNK * GROUP)
    g_e = gqt // (NCHUNK * GROUP)
    q_e = (gqt // GROUP) % NCHUNK
    t_e = gqt % GROUP
    in_span = seg_off[g_e, q_e, t_e] + rank_in_seg
    raw_blk = in_span // P
    r1 = (raw_blk * P < seg_off[g_e, q_e, t_e]).astype(np.int64)
    new_blk = np.empty(len(key_s), dtype=np.int64)
    for g in range(NG):
        for q in range(NCHUNK):
            m = (g_e == g) & (q_e == q)
            new_blk[m] = perm_maps[g][q][raw_blk[m]]
    pos = (span_blk_base[g_e, q_e] + new_blk) * P + (in_span % P)
    cc = key_s // (NG * NCHUNK * GROUP)
    flat = cc * NIDX + pos

    idx_all = np.zeros(NCORES * NIDX, dtype=np.int16)
    idx_all[flat] = (src_s - q_s * CHUNK).astype(np.int16)
    enc_all = np.full(NCORES * NIDX, 1000.0, dtype=np.float16)
    enc_all[flat] = (dst_s % P + 128 * r1).astype(np.float16)
    nrm_all = np.zeros(NCORES * NIDX, dtype=np.float16)
    nrm_all[flat] = nrm_s.astype(np.float16)

    idx_tiles, enc_tiles, norm_tiles = [], [], []
    for c in range(NCORES):
        iv = idx_all[c * NIDX:(c + 1) * NIDX]
        idx_tiles.append(np.ascontiguousarray(
            np.tile(iv.reshape(NIDX // 16, 16).T, (8, 1))))
        dv = enc_all[c * NIDX:(c + 1) * NIDX]
        enc_tiles.append(np.ascontiguousarray(dv.reshape(NBLK, P).T))
        nv = nrm_all[c * NIDX:(c + 1) * NIDX]
        norm_tiles.append(np.ascontiguousarray(nv.reshape(NBLK, P).T))

    sched = {
        "SB": SB, "nb1": nb1, "t0": t0_lists,
        "NBLK": NBLK, "SMAX": int(SB.max()), "NB1MAX": int(nb1.max()),
    }
    return idx_tiles, enc_tiles, norm_tiles, dis2_tiles, sched


def _build_program(sched):
    _install_tile_patches()
    SB = sched["SB"]
    nb1 = sched["nb1"]
    t0_lists = sched["t0"]
    NBLK = sched["NBLK"]
    NIDX = NBLK * P
    SMAX = sched["SMAX"]
    NB1MAX = max(sched["NB1MAX"], 1)

    nc = bass.Bass(dynamic_dma_scratch_size=65536)
    x_d = nc.dram_tensor("x16", [VPAD, D], F16, kind="ExternalInput")
    W_d = nc.dram_tensor("W", [D, D], F32, kind="ExternalInput")
    b_d = nc.dram_tensor("b", [1, D], F32, kind="ExternalInput")
    idx_d = nc.dram_tensor("idx16", [P, NIDX // 16], I16, kind="ExternalInput")
    dst_d = nc.dram_tensor("dstloc", [P, NBLK], F16, kind="ExternalInput")
    nrm_d = nc.dram_tensor("normc", [P, NBLK], F16, kind="ExternalInput")
    dis2_d = nc.dram_tensor("dis2", [P, NT_SH], F32, kind="ExternalInput")
    xself_d = nc.dram_tensor("xself", [SH, D], F16, kind="ExternalInput")
    iotar_d = nc.dram_tensor("iota_rep", [P, P * SMAX], F16, kind="ExternalInput")
    iotar1_d = nc.dram_tensor("iota_rep1", [P, P * NB1MAX], F16, kind="ExternalInput")
    ones_d = nc.dram_tensor("ones", [1, P], F16, kind="ExternalInput")
    out_d = nc.dram_tensor("out", [SH, D], F32, kind="ExternalOutput")

    with tile.TileContext(nc) as tc:
        with (
            tc.tile_pool(name="const", bufs=1) as cp,
            tc.tile_pool(name="sgx", bufs=3) as sgx,
            tc.tile_pool(name="sg16", bufs=3) as sg16,
            tc.tile_pool(name="sm", bufs=2) as sm,
            tc.tile_pool(name="sxt", bufs=2) as sxt,
            tc.tile_pool(name="sacc", bufs=2) as sacc,
            tc.tile_pool(name="sout", bufs=2) as sout,
            tc.tile_pool(name="pacc", bufs=1, space="PSUM") as pacc,
            tc.tile_pool(name="pout", bufs=1, space="PSUM") as pout,
        ):
            idx_sb = cp.tile([P, NIDX // 16], I16)
            nc.sync.dma_start(out=idx_sb[:], in_=idx_d[:])
            W_sb = cp.tile([P, P], F32)
            nc.sync.dma_start(out=W_sb[:], in_=W_d[:])
            W16 = cp.tile([P, P], F16)
            nc.vector.tensor_copy(out=W16[:], in_=W_sb[:])
            b_sb = cp.tile([1, P], F32)
            nc.sync.dma_start(out=b_sb[:], in_=b_d[:])
            b16 = cp.tile([1, P], F16)
            nc.vector.tensor_copy(out=b16[:], in_=b_sb[:])
            ones16 = cp.tile([1, P], F16)
            nc.sync.dma_start(out=ones16[:], in_=ones_d[:])
            zero16 = cp.tile([1, P], F16)
            nc.vector.memset(zero16[:], 0.0)
            ident32 = cp.tile([P, P], F32)
            make_identity(nc, ident32[:])
            ident16 = cp.tile([P, P], F16)
            nc.vector.tensor_copy(out=ident16[:], in_=ident32[:])
            iotar_sb = cp.tile([P, P * SMAX], F16)
            nc.sync.dma_start(out=iotar_sb[:], in_=iotar_d[:])
            iotar1_sb = cp.tile([P, P * NB1MAX], F16)
            nc.sync.dma_start(out=iotar1_sb[:], in_=iotar1_d[:])
            dst_sb = cp.tile([P, NBLK], F16)
            nc.sync.dma_start(out=dst_sb[:], in_=dst_d[:])
            nrm_sb = cp.tile([P, NBLK], F16)
            nc.sync.dma_start(out=nrm_sb[:], in_=nrm_d[:])
            dis2_sb = cp.tile([P, NT_SH], F32)
            nc.sync.dma_start(out=dis2_sb[:], in_=dis2_d[:])

            nreg = nc.gpsimd.alloc_register()
            nc.gpsimd.load_library(library_config.mlp)

            bi = 0
            for g in range(NG):
                accs = [pacc.tile([P, P], F32, space="PSUM", tag=f"acc{tg}",
                                  name=f"acc{tg}")[:] for tg in range(GROUP)]
                # self-loop contribution: batched x-tile load, per-tile
                # cast*dis2 on ACT, matmul vs identity (opens each acc).
                xg16 = sxt.tile([P, GROUP * P], F16, tag="xg16")
                nc.sync.dma_start(
                    out=xg16[:].rearrange("p (j f) -> p j f", f=P),
                    in_=xself_d[g * GROUP * P:(g + 1) * GROUP * P, :].rearrange(
                        "(j p) f -> p j f", p=P))
                for tg in range(GROUP):
                    t = g * GROUP + tg
                    diag = sacc.tile([P, P], F16, tag=f"dg{tg % 2}",
                                     name=f"diag{tg}")
                    nc.vector.tensor_scalar(
                        out=diag[:], in0=ident16[:],
                        scalar1=dis2_sb[:, t:t + 1], scalar2=None,
                        op0=mybir.AluOpType.mult)
                    nc.tensor.matmul(out=accs[tg],
                                     lhsT=xg16[:, tg * P:(tg + 1) * P],
                                     rhs=diag[:], start=True, stop=False)
                # last edge-matmul per tile gets stop=True
                emis = [[] for _ in range(GROUP)]
                seq = 0
                for q in range(NCHUNK):
                    sb = int(SB[g, q])
                    nb = int(nb1[g, q])
                    t0l = t0_lists[g][q]
                    for j in range(sb):
                        emis[t0l[j]].append(seq)
                        seq += 1
                        if j >= sb - nb:
                            emis[t0l[j] + 1].append(seq)
                            seq += 1
                lasts = {max(e) for e in emis if e}
                empty_tiles = [tg for tg, e in enumerate(emis) if not e]

                seq = 0
                for q in range(NCHUNK):
                    sb = int(SB[g, q])
                    nb = int(nb1[g, q])
                    t0l = t0_lists[g][q]
                    gbuf = sgx.tile([P, SMAX * P], F16, tag="g32")
                    nc.gpsimd.reg_mov(nreg, sb * P)
                    nc.gpsimd.dma_gather(
                        out_ap=gbuf[:, :sb * P].rearrange("p (j f) -> p j f", f=P),
                        in_ap=x_d[q * CHUNK:(q + 1) * CHUNK, :],
                        idxs_ap=idx_sb[:, bi * 8:(bi + sb) * 8],
                        num_idxs=sb * P,
                        num_idxs_reg=nreg,
                        elem_size=P,
                        single_packet=False,
                    )
                    g16 = gbuf
                    meq = sm.tile([P, P * SMAX], F16, tag="meq")
                    nc.vector.tensor_tensor(
                        out=meq[:].rearrange("p (d j) -> p d j", j=SMAX)[:, :, :sb],
                        in0=dst_sb[:, bi:bi + sb].rearrange(
                            "p (one j) -> p one j", one=1).to_broadcast([P, P, sb]),
                        in1=iotar_sb[:].rearrange(
                            "p (d j) -> p d j", j=SMAX)[:, :, :sb],
                        op=mybir.AluOpType.is_equal,
                    )
                    mn = sm.tile([P, P * SMAX], F16, tag="mn")
                    nc.vector.tensor_tensor(
                        out=mn[:].rearrange("p (d j) -> p d j", j=SMAX)[:, :, :sb],
                        in0=meq[:].rearrange("p (d j) -> p d j", j=SMAX)[:, :, :sb],
                        in1=nrm_sb[:, bi:bi + sb].rearrange(
                            "p (one j) -> p one j", one=1).to_broadcast([P, P, sb]),
                        op=mybir.AluOpType.mult,
                    )
                    mnv = mn[:].rearrange("p (d j) -> p d j", j=SMAX)
                    if nb > 0:
                        b0 = bi + sb - nb
                        m1q = sm.tile([P, P * NB1MAX], F16, tag="m1q")
                        nc.vector.tensor_tensor(
                            out=m1q[:].rearrange(
                                "p (d j) -> p d j", j=NB1MAX)[:, :, :nb],
                            in0=dst_sb[:, b0:b0 + nb].rearrange(
                                "p (one j) -> p one j", one=1).to_broadcast([P, P, nb]),
                            in1=iotar1_sb[:].rearrange(
                                "p (d j) -> p d j", j=NB1MAX)[:, :, :nb],
                            op=mybir.AluOpType.is_equal,
                        )
                        mn1 = sm.tile([P, P * NB1MAX], F16, tag="mn1")
                        nc.vector.tensor_tensor(
                            out=mn1[:].rearrange(
                                "p (d j) -> p d j", j=NB1MAX)[:, :, :nb],
                            in0=m1q[:].rearrange(
                                "p (d j) -> p d j", j=NB1MAX)[:, :, :nb],
                            in1=nrm_sb[:, b0:b0 + nb].rearrange(
                                "p (one j) -> p one j", one=1).to_broadcast([P, P, nb]),
                            op=mybir.AluOpType.mult,
                        )
                        mn1v = mn1[:].rearrange("p (d j) -> p d j", j=NB1MAX)
                    for j in range(sb):
                        tgt = t0l[j]
                        nc.tensor.matmul(
                            out=accs[tgt],
                            lhsT=g16[:, j * P:(j + 1) * P],
                            rhs=mnv[:, :, j],
                            start=False,
                            stop=(seq in lasts),
                        )
                        seq += 1
                        if j >= sb - nb:
                            k = j - (sb - nb)
                            nc.tensor.matmul(
                                out=accs[tgt + 1],
                                lhsT=g16[:, j * P:(j + 1) * P],
                                rhs=mn1v[:, :, k],
                                start=False,
                                stop=(seq in lasts),
                            )
                            seq += 1
                    bi += sb
                # tiles with no edge blocks: close the accumulation with a
                # zero-contribution matmul (rank-1 x ones row of zeros is
                # avoided by reusing the bias trick: 1-row matmul of zeros).
                for tg in empty_tiles:
                    nc.tensor.matmul(out=accs[tg], lhsT=ones16[0:1, :],
                                     rhs=zero16[0:1, :], start=False, stop=True)
                obuf = sout.tile([P, GROUP * P], F32, tag="o")
                for tg in range(GROUP):
                    accT = sacc.tile([P, P], F16, tag=f"at{tg % 2}",
                                     name=f"accT{tg}")
                    nc.vector.tensor_copy(out=accT[:], in_=accs[tg])
                    po = pout.tile([P, P], F32, space="PSUM", tag="po")
                    nc.tensor.matmul(out=po[:], lhsT=accT[:], rhs=W16[:],
                                     start=True, stop=False)
                    nc.tensor.matmul(out=po[:], lhsT=ones16[0:1, :], rhs=b16[0:1, :],
                                     start=False, stop=True)
                    nc.vector.tensor_scalar(
                        out=obuf[:, tg * P:(tg + 1) * P], in0=po[:], scalar1=0.0,
                        scalar2=None, op0=mybir.AluOpType.max)
                nc.sync.dma_start(
                    out=out_d[g * GROUP * P:(g + 1) * GROUP * P, :].rearrange(
                        "(j p) f -> p j f", p=P),
                    in_=obuf[:].rearrange("p (j f) -> p j f", f=P))
    lower_extended_insts(nc)
    return nc


def _make_inputs(x, W, b, idx_tiles, enc_tiles, norm_tiles, dis2_tiles, sched, c):
    SMAX = sched["SMAX"]
    NB1MAX = max(sched["NB1MAX"], 1)
    xpad = np.zeros((VPAD, D), dtype=np.float32)
    xpad[:N] = np.asarray(x, dtype=np.float32)
    x16 = np.ascontiguousarray(xpad.astype(np.float16))
    iota_rep = np.ascontiguousarray(
        np.repeat(np.arange(P, dtype=np.float16), SMAX)[None, :].repeat(P, 0))
    iota_rep1 = np.ascontiguousarray(
        np.repeat(np.arange(P, dtype=np.float32) + 128.0,
                  NB1MAX).astype(np.float16)[None, :].repeat(P, 0))
    return {
        "x16": x16,
        "xself": np.ascontiguousarray(x16[c * SH:(c + 1) * SH]),
        "W": np.asarray(W, dtype=np.float32),
        "b": np.asarray(b).reshape(1, D).astype(np.float32),
        "idx16": idx_tiles[c],
        "dstloc": enc_tiles[c],
        "normc": norm_tiles[c],
        "dis2": dis2_tiles[c],
        "iota_rep": iota_rep,
        "iota_rep1": iota_rep1,
        "ones": np.ones((1, P), dtype=np.float16),
    }


def kernel(x, edge_index, W, b):
    x = np.asarray(x)
    edge_index = np.asarray(edge_index)

    idx_tiles, enc_tiles, norm_tiles, dis2_tiles, sched = _host_prep(edge_index)
    nc = _build_program(sched)
    in_maps = [_make_inputs(x, W, b, idx_tiles, enc_tiles, norm_tiles,
                            dis2_tiles, sched, c)
               for c in range(NCORES)]
    res = run_bass_kernel_spmd(nc, in_maps, core_ids=list(range(NCORES)), trace=False)
    out = np.concatenate([res.results[c]["out"] for c in range(NCORES)], axis=0)
    return out[:N]


# revision 13
# speedup vs baseline: 1.0135x; 1.0135x over previous
"""GCN layer kernel for trn2, 8-core SPMD — v5.1.

out = relu(D^-1/2 (A+I) D^-1/2 X W + b) = relu((A_hat X) W + b)

Architecture (per core, dst-sharded; GROUP=7 dst tiles per gather group, src
space in 4 chunks of 25088 rows for int16 gather indices):
- Self-loops are excluded from the edge list. Their contribution
  (dis_d^2 * x_d) enters per tile as matmul(lhsT=cast(x_tile)*dis^2,
  rhs=identity): the ACT cast's per-partition `scale` applies dis^2 free,
  and the x tile is a cheap sequential load. This avoids the self-loop
  block inflating every (tile,chunk) segment's max-over-cores count.
- Edge segments per (tile,chunk) are padded to the max count over the 8
  cores (core-uniform schedule), not to x128; blocks may cross tile
  boundaries. A block containing a transition carries rank-1 edges
  (enc = dstloc + 128); those blocks are permuted to the span end so the
  rank-1 one-hot builds are small contiguous flip-layout DVE passes.
- One dma_gather per (group, chunk) fetches raw fp32 x rows (512B each).
- ACT casts gathers fp32->fp16; DVE builds M_norm = is_equal * norm in the
  flip layout m[p, d*SMAX+j] (2x mode); PE accumulates
  accT[f,d] += sum_e norm_e x[src_e]^T via matmul(lhsT=G_j, rhs=M[:,:,j]).
- Per tile: ACT copies accT to fp16; PE applies W + rank-1 bias; ACT relu;
  one batched store per group.
"""
import numpy as np

import concourse.bass as bass
import concourse.mybir as mybir
import concourse.tile as tile
from concourse import library_config
from concourse.bass_utils import run_bass_kernel_spmd
from concourse.library_overlay import lower_extended_insts
from concourse.masks import make_identity

# ---- walrus workaround: <=1 sync wait per instruction ----
def _install_tile_patches():
    K = 1

    def _split_waits(tc, ordered):
        nc = tc.nc
        for insts in ordered.values():
            out = []
            for inst in insts:
                si = inst.sync_info
                waits = list(si.on_wait) if si is not None else []
                if len(waits) > K:
                    for i in range(0, len(waits) - K, K):
                        out.append(mybir.InstNoOp(
                            name=nc.get_next_instruction_name(),
                            engine=inst.engine, ins=[], outs=[],
                            sync_info=mybir.SyncInfo(on_wait=waits[i:i+K], on_update=[]),
                        ))
                    inst.sync_info = mybir.SyncInfo(
                        on_wait=waits[len(waits) - K:], on_update=list(si.on_update))
                out.append(inst)
            insts[:] = out

    if getattr(tile.TileContext, "_ant_waitsplit", False):
        return
    orig_lower = tile.TileContext._lower_ordered_insts

    def lower(self, ordered):
        _split_waits(self, ordered)
        return orig_lower(self, ordered)

    def drain(self, tick_clock, wait_clock):
        probe = self.nc.sync.nop(nofuse=True, hint="pre_drain_waits")
        wait_clock.add_sem_waits(probe.ins, tile.ScopedClock({None: tick_clock.global_clock}))
        waits = list(probe.ins.sync_info.on_wait)
        probe.ins.sync_info = mybir.SyncInfo(on_wait=waits[:K], on_update=[])
        for i in range(K, len(waits), K):
            n = self.nc.sync.nop(nofuse=True, hint="pre_drain_waits")
            n.ins.sync_info = mybir.SyncInfo(on_wait=waits[i:i+K], on_update=[])
        self.nc.sync.drain()
        self.nc.all_engine_barrier()
        popped = self.nc._tile_sem_poison_stack.pop()
        assert popped is self._sem_poison
        self.nc.clear_and_free_semaphores(list(self.sems.allocated().values()))
        self.nc.all_engine_barrier()

    tile.TileContext._lower_ordered_insts = lower
    tile.TileContext._drain_and_barrier = drain
    tile.TileContext._ant_waitsplit = True



N = 100000
D = 128
NCORES = 8
P = 128
VPAD = 100352
SH = VPAD // NCORES      # 12544
NT_SH = SH // P          # 98
NCHUNK = 4
CHUNK = VPAD // NCHUNK   # 25088
GROUP = 7
NG = NT_SH // GROUP      # 14

F16 = mybir.dt.float16
F32 = mybir.dt.float32
I16 = mybir.dt.int16


def _host_prep(edge_index):
    """Returns (idx_tiles, enc_tiles, norm_tiles, dis2_tiles, sched)."""
    src = edge_index[0].astype(np.int64)
    dst = edge_index[1].astype(np.int64)

    deg = np.bincount(dst, minlength=VPAD).astype(np.float64) + 1.0
    dis = 1.0 / np.sqrt(deg)
    norm = (dis[src] * dis[dst]).astype(np.float32)
    dis2 = (1.0 / deg).astype(np.float32)
    dis2_tiles = [
        np.ascontiguousarray(
            dis2[c * SH:(c + 1) * SH].reshape(NT_SH, P).T.astype(np.float32))
        for c in range(NCORES)
    ]

    c_of = dst // SH
    t_loc = (dst % SH) // P
    g_of = t_loc // GROUP
    tg_of = t_loc % GROUP
    q_of = src // CHUNK
    key = (((c_of * NG + g_of) * NCHUNK) + q_of) * GROUP + tg_of
    NKEYS = NCORES * NG * NCHUNK * GROUP

    order = np.lexsort((src, key))
    src_s = src[order]
    dst_s = dst[order]
    nrm_s = norm[order]
    key_s = key[order]
    q_s = q_of[order]

    cnt = np.bincount(key_s, minlength=NKEYS).reshape(NCORES, NG, NCHUNK, GROUP)
    L = np.maximum(cnt.max(axis=0), P)            # [NG, NCHUNK, GROUP]
    seg_off = np.zeros((NG, NCHUNK, GROUP), dtype=np.int64)
    seg_off[:, :, 1:] = np.cumsum(L, axis=2)[:, :, :-1]
    span_raw = L.sum(axis=2)
    SB = (span_raw + P - 1) // P

    nb1 = np.zeros((NG, NCHUNK), dtype=np.int64)
    t0_lists = [[None] * NCHUNK for _ in range(NG)]
    perm_maps = [[None] * NCHUNK for _ in range(NG)]
    for g in range(NG):
        for q in range(NCHUNK):
            sb = int(SB[g, q])
            offs = seg_off[g, q]
            raw_t0 = np.searchsorted(offs, np.arange(sb) * P, side="right") - 1
            bset = sorted({int(offs[t]) // P for t in range(1, GROUP)
                           if offs[t] % P != 0})
            nb = len(bset)
            nonb = [bk for bk in range(sb) if bk not in bset]
            old_order = nonb + bset
            perm = np.empty(sb, dtype=np.int64)
            for newj, oldb in enumerate(old_order):
                perm[oldb] = newj
            nb1[g, q] = nb
            t0_lists[g][q] = [int(raw_t0[oldb]) for oldb in old_order]
            perm_maps[g][q] = perm

    span_blk_base = np.zeros((NG, NCHUNK), dtype=np.int64)
    span_blk_base.reshape(-1)[1:] = np.cumsum(SB.reshape(-1))[:-1]
    NBLK = int(SB.sum())
    NIDX = NBLK * P

    seg_start = np.zeros(NKEYS + 1, dtype=np.int64)
    np.cumsum(np.bincount(key_s, minlength=NKEYS), out=seg_start[1:])
    rank_in_seg = np.arange(len(key_s), dtype=np.int64) - seg_start[key_s]
    gqt = key_s % (NG * NCHUNK * GROUP)
    g_e = gqt // (NCHUNK * GROUP)
    q_e = (gqt // GROUP) % NCHUNK
    t_e = gqt % GROUP
    in_span = seg_off[g_e, q_e, t_e] + rank_in_seg
    raw_blk = in_span // P
    r1 = (raw_blk * P < seg_off[g_e, q_e, t_e]).astype(np.int64)
    new_blk = np.empty(len(key_s), dtype=np.int64)
    for g in range(NG):
        for q in range(NCHUNK):
            m = (g_e == g) & (q_e == q)
            new_blk[m] = perm_maps[g][q][raw_blk[m]]
    pos = (span_blk_base[g_e, q_e] + new_blk) * P + (in_span % P)
    cc = key_s // (NG * NCHUNK * GROUP)
    flat = cc * NIDX + pos

    idx_all = np.zeros(NCORES * NIDX, dtype=np.int16)
    idx_all[flat] = (src_s - q_s * CHUNK).astype(np.int16)
    enc_all = np.full(NCORES * NIDX, 1000.0, dtype=np.float16)
    enc_all[flat] = (dst_s % P + 128 * r1).astype(np.float16)
    nrm_all = np.zeros(NCORES * NIDX, dtype=np.float16)
    nrm_all[flat] = nrm_s.astype(np.float16)

    idx_tiles, enc_tiles, norm_tiles = [], [], []
    for c in range(NCORES):
        iv = idx_all[c * NIDX:(c + 1) * NIDX]
        idx_tiles.append(np.ascontiguousarray(
            np.tile(iv.reshape(NIDX // 16, 16).T, (8, 1))))
        dv = enc_all[c * NIDX:(c + 1) * NIDX]
        enc_tiles.append(np.ascontiguousarray(dv.reshape(NBLK, P).T))
        nv = nrm_all[c * NIDX:(c + 1) * NIDX]
        norm_tiles.append(np.ascontiguousarray(nv.reshape(NBLK, P).T))

    sched = {
        "SB": SB, "nb1": nb1, "t0": t0_lists,
        "NBLK": NBLK, "SMAX": int(SB.max()), "NB1MAX": int(nb1.max()),
    }
    return idx_tiles, enc_tiles, norm_tiles, dis2_tiles, sched


def _build_program(sched):
    _install_tile_patches()
    SB = sched["SB"]
    nb1 = sched["nb1"]
    t0_lists = sched["t0"]
    NBLK = sched["NBLK"]
    NIDX = NBLK * P
    SMAX = sched["SMAX"]
    NB1MAX = max(sched["NB1MAX"], 1)

    nc = bass.Bass()
    x_d = nc.dram_tensor("x16", [VPAD, D], F16, kind="ExternalInput")
    W_d = nc.dram_tensor("W", [D, D], F32, kind="ExternalInput")
    b_d = nc.dram_tensor("b", [1, D], F32, kind="ExternalInput")
    idx_d = nc.dram_tensor("idx16", [P, NIDX // 16], I16, kind="ExternalInput")
    dst_d = nc.dram_tensor("dstloc", [P, NBLK], F16, kind="ExternalInput")
    nrm_d = nc.dram_tensor("normc", [P, NBLK], F16, kind="ExternalInput")
    dis2_d = nc.dram_tensor("dis2", [P, NT_SH], F32, kind="ExternalInput")
    xself_d = nc.dram_tensor("xself", [SH, D], F16, kind="ExternalInput")
    iotar_d = nc.dram_tensor("iota_rep", [P, P * SMAX], F16, kind="ExternalInput")
    iotar1_d = nc.dram_tensor("iota_rep1", [P, P * NB1MAX], F16, kind="ExternalInput")
    ones_d = nc.dram_tensor("ones", [1, P], F16, kind="ExternalInput")
    out_d = nc.dram_tensor("out", [SH, D], F32, kind="ExternalOutput")

    with tile.TileContext(nc) as tc:
        with (
            tc.tile_pool(name="const", bufs=1) as cp,
            tc.tile_pool(name="sgx", bufs=3) as sgx,
            tc.tile_pool(name="sg16", bufs=3) as sg16,
            tc.tile_pool(name="sm", bufs=2) as sm,
            tc.tile_pool(name="sxt", bufs=2) as sxt,
            tc.tile_pool(name="sacc", bufs=2) as sacc,
            tc.tile_pool(name="sout", bufs=2) as sout,
            tc.tile_pool(name="pacc", bufs=1, space="PSUM") as pacc,
            tc.tile_pool(name="pout", bufs=1, space="PSUM") as pout,
        ):
            idx_sb = cp.tile([P, NIDX // 16], I16)
            nc.sync.dma_start(out=idx_sb[:], in_=idx_d[:])
            W_sb = cp.tile([P, P], F32)
            nc.sync.dma_start(out=W_sb[:], in_=W_d[:])
            W16 = cp.tile([P, P], F16)
            nc.vector.tensor_copy(out=W16[:], in_=W_sb[:])
            b_sb = cp.tile([1, P], F32)
            nc.sync.dma_start(out=b_sb[:], in_=b_d[:])
            b16 = cp.tile([1, P], F16)
            nc.vector.tensor_copy(out=b16[:], in_=b_sb[:])
            ones16 = cp.tile([1, P], F16)
            nc.sync.dma_start(out=ones16[:], in_=ones_d[:])
            zero16 = cp.tile([1, P], F16)
            nc.vector.memset(zero16[:], 0.0)
            ident32 = cp.tile([P, P], F32)
            make_identity(nc, ident32[:])
            ident16 = cp.tile([P, P], F16)
            nc.vector.tensor_copy(out=ident16[:], in_=ident32[:])
            iotar_sb = cp.tile([P, P * SMAX], F16)
            nc.sync.dma_start(out=iotar_sb[:], in_=iotar_d[:])
            iotar1_sb = cp.tile([P, P * NB1MAX], F16)
            nc.sync.dma_start(out=iotar1_sb[:], in_=iotar1_d[:])
            dst_sb = cp.tile([P, NBLK], F16)
            nc.sync.dma_start(out=dst_sb[:], in_=dst_d[:])
            nrm_sb = cp.tile([P, NBLK], F16)
            nc.sync.dma_start(out=nrm_sb[:], in_=nrm_d[:])
            dis2_sb = cp.tile([P, NT_SH], F32)
            nc.sync.dma_start(out=dis2_sb[:], in_=dis2_d[:])

            nreg = nc.gpsimd.alloc_register()
            nc.gpsimd.load_library(library_config.mlp)

            bi = 0
            for g in range(NG):
                accs = [pacc.tile([P, P], F32, space="PSUM", tag=f"acc{tg}",
                                  name=f"acc{tg}")[:] for tg in range(GROUP)]
                # self-loop contribution: batched x-tile load, per-tile
                # cast*dis2 on ACT, matmul vs identity (opens each acc).
                xg16 = sxt.tile([P, GROUP * P], F16, tag="xg16")
                nc.sync.dma_start(
                    out=xg16[:].rearrange("p (j f) -> p j f", f=P),
                    in_=xself_d[g * GROUP * P:(g + 1) * GROUP * P, :].rearrange(
                        "(j p) f -> p j f", p=P))
                for tg in range(GROUP):
                    t = g * GROUP + tg
                    diag = sacc.tile([P, P], F16, tag=f"dg{tg % 2}",
                                     name=f"diag{tg}")
                    nc.vector.tensor_scalar(
                        out=diag[:], in0=ident16[:],
                        scalar1=dis2_sb[:, t:t + 1], scalar2=None,
                        op0=mybir.AluOpType.mult)
                    nc.tensor.matmul(out=accs[tg],
                                     lhsT=xg16[:, tg * P:(tg + 1) * P],
                                     rhs=diag[:], start=True, stop=False)
                # last edge-matmul per tile gets stop=True
                emis = [[] for _ in range(GROUP)]
                seq = 0
                for q in range(NCHUNK):
                    sb = int(SB[g, q])
                    nb = int(nb1[g, q])
                    t0l = t0_lists[g][q]
                    for j in range(sb):
                        emis[t0l[j]].append(seq)
                        seq += 1
                        if j >= sb - nb:
                            emis[t0l[j] + 1].append(seq)
                            seq += 1
                lasts = {max(e) for e in emis if e}
                empty_tiles = [tg for tg, e in enumerate(emis) if not e]

                seq = 0
                for q in range(NCHUNK):
                    sb = int(SB[g, q])
                    nb = int(nb1[g, q])
                    t0l = t0_lists[g][q]
                    gbuf = sgx.tile([P, SMAX * P], F16, tag="g32")
                    nc.gpsimd.reg_mov(nreg, sb * P)
                    nc.gpsimd.dma_gather(
                        out_ap=gbuf[:, :sb * P].rearrange("p (j f) -> p j f", f=P),
                        in_ap=x_d[q * CHUNK:(q + 1) * CHUNK, :],
                        idxs_ap=idx_sb[:, bi * 8:(bi + sb) * 8],
                        num_idxs=sb * P,
                        num_idxs_reg=nreg,
                        elem_size=P,
                        single_packet=False,
                    )
                    g16 = gbuf
                    meq = sm.tile([P, P * SMAX], F16, tag="meq")
                    nc.vector.tensor_tensor(
                        out=meq[:].rearrange("p (d j) -> p d j", j=SMAX)[:, :, :sb],
                        in0=dst_sb[:, bi:bi + sb].rearrange(
                            "p (one j) -> p one j", one=1).to_broadcast([P, P, sb]),
                        in1=iotar_sb[:].rearrange(
                            "p (d j) -> p d j", j=SMAX)[:, :, :sb],
                        op=mybir.AluOpType.is_equal,
                    )
                    mn = sm.tile([P, P * SMAX], F16, tag="mn")
                    nc.vector.tensor_tensor(
                        out=mn[:].rearrange("p (d j) -> p d j", j=SMAX)[:, :, :sb],
                        in0=meq[:].rearrange("p (d j) -> p d j", j=SMAX)[:, :, :sb],
                        in1=nrm_sb[:, bi:bi + sb].rearrange(
                            "p (one j) -> p one j", one=1).to_broadcast([P, P, sb]),
                        op=mybir.AluOpType.mult,
                    )
                    mnv = mn[:].rearrange("p (d j) -> p d j", j=SMAX)
                    if nb > 0:
                        b0 = bi + sb - nb
                        m1q = sm.tile([P, P * NB1MAX], F16, tag="m1q")
                        nc.vector.tensor_tensor(
                            out=m1q[:].rearrange(
                                "p (d j) -> p d j", j=NB1MAX)[:, :, :nb],
                            in0=dst_sb[:, b0:b0 + nb].rearrange(
                                "p (one j) -> p one j", one=1).to_broadcast([P, P, nb]),
                            in1=iotar1_sb[:].rearrange(
                                "p (d j) -> p d j", j=NB1MAX)[:, :, :nb],
                            op=mybir.AluOpType.is_equal,
                        )
                        mn1 = sm.tile([P, P * NB1MAX], F16, tag="mn1")
                        nc.vector.tensor_tensor(
                            out=mn1[:].rearrange(
                                "p (d j) -> p d j", j=NB1MAX)[:, :, :nb],
                            in0=m1q[:].rearrange(
                                "p (d j) -> p d j", j=NB1MAX)[:, :, :nb],
                            in1=nrm_sb[:, b0:b0 + nb].rearrange(
                                "p (one j) -> p one j", one=1).to_broadcast([P, P, nb]),
                            op=mybir.AluOpType.mult,
                        )
                        mn1v = mn1[:].rearrange("p (d j) -> p d j", j=NB1MAX)
                    for j in range(sb):
                        tgt = t0l[j]
                        nc.tensor.matmul(
                            out=accs[tgt],
                            lhsT=g16[:, j * P:(j + 1) * P],
                            rhs=mnv[:, :, j],
                            start=False,
                            stop=(seq in lasts),
                        )
                        seq += 1
                        if j >= sb - nb:
                            k = j - (sb - nb)
                            nc.tensor.matmul(
                                out=accs[tgt + 1],
                                lhsT=g16[:, j * P:(j + 1) * P],
                                rhs=mn1v[:, :, k],
                                start=False,
                                stop=(seq in lasts),
                            )
                            seq += 1
                    bi += sb
                # tiles with no edge blocks: close the accumulation with a
                # zero-contribution matmul (rank-1 x ones row of zeros is
                # avoided by reusing the bias trick: 1-row matmul of zeros).
                for tg in empty_tiles:
                    nc.tensor.matmul(out=accs[tg], lhsT=ones16[0:1, :],
                                     rhs=zero16[0:1, :], start=False, stop=True)
                obuf = sout.tile([P, GROUP * P], F32, tag="o")
                for tg in range(GROUP):
                    accT = sacc.tile([P, P], F16, tag=f"at{tg % 2}",
                                     name=f"accT{tg}")
                    nc.vector.tensor_copy(out=accT[:], in_=accs[tg])
                    po = pout.tile([P, P], F32, space="PSUM", tag="po")
                    nc.tensor.matmul(out=po[:], lhsT=accT[:], rhs=W16[:],
                                     start=True, stop=False)
                    nc.tensor.matmul(out=po[:], lhsT=ones16[0:1, :], rhs=b16[0:1, :],
                                     start=False, stop=True)
                    nc.vector.tensor_scalar(
                        out=obuf[:, tg * P:(tg + 1) * P], in0=po[:], scalar1=0.0,
                        scalar2=None, op0=mybir.AluOpType.max)
                nc.sync.dma_start(
                    out=out_d[g * GROUP * P:(g + 1) * GROUP * P, :].rearrange(
                        "(j p) f -> p j f", p=P),
                    in_=obuf[:].rearrange("p (j f) -> p j f", f=P))
    lower_extended_insts(nc)
    return nc


def _make_inputs(x, W, b, idx_tiles, enc_tiles, norm_tiles, dis2_tiles, sched, c):
    SMAX = sched["SMAX"]
    NB1MAX = max(sched["NB1MAX"], 1)
    xpad = np.zeros((VPAD, D), dtype=np.float32)
    xpad[:N] = np.asarray(x, dtype=np.float32)
    x16 = np.ascontiguousarray(xpad.astype(np.float16))
    iota_rep = np.ascontiguousarray(
        np.repeat(np.arange(P, dtype=np.float16), SMAX)[None, :].repeat(P, 0))
    iota_rep1 = np.ascontiguousarray(
        np.repeat(np.arange(P, dtype=np.float32) + 128.0,
                  NB1MAX).astype(np.float16)[None, :].repeat(P, 0))
    return {
        "x16": x16,
        "xself": np.ascontiguousarray(x16[c * SH:(c + 1) * SH]),
        "W": np.asarray(W, dtype=np.float32),
        "b": np.asarray(b).reshape(1, D).astype(np.float32),
        "idx16": idx_tiles[c],
        "dstloc": enc_tiles[c],
        "normc": norm_tiles[c],
        "dis2": dis2_tiles[c],
        "iota_rep": iota_rep,
        "iota_rep1": iota_rep1,
        "ones": np.ones((1, P), dtype=np.float16),
    }


def kernel(x, edge_index, W, b):
    x = np.asarray(x)
    edge_index = np.asarray(edge_index)

    idx_tiles, enc_tiles, norm_tiles, dis2_tiles, sched = _host_prep(edge_index)
    nc = _build_program(sched)
    in_maps = [_make_inputs(x, W, b, idx_tiles, enc_tiles, norm_tiles,
                            dis2_tiles, sched, c)
               for c in range(NCORES)]
    res = run_bass_kernel_spmd(nc, in_maps, core_ids=list(range(NCORES)), trace=False)
    out = np.concatenate([res.results[c]["out"] for c in range(NCORES)], axis=0)
    return out[:N]
